# revision 17
# baseline (speedup 1.0000x reference)
"""Trainium2 Bass kernel for autoregressive masked-conv sampling.

Model: type-A masked 5x5 conv [n,C,H,W] -> [n,K,C,H,W] logits; per-pixel
raster-order categorical sampling over K with jax threefry gumbel noise.

Strategy:
  - Gumbel noise is data-independent => precompute on host CPU with jax,
    bit-matching jax.random.categorical's internals.
  - Data-parallel over batch n=64 across 8 cores (8 samples/core).
  - On device, per pixel i: gather 3x5 (x3 in-ch) padded window -> [45,8]
    patchT, 3 matmuls (one per conv in-group c_out) -> PSUM [24,128]
    (partition p = c_out*8 + n_local), add noise slice, DVE max/max_index
    argmax over K, cast to f32, scatter the sampled value back into the
    padded image buffer. 144 strictly sequential steps.
  - Conditioned pixels (cond >= 0) are forced via +/-1e30 noise so the
    argmax returns the conditioned (integer) value; final exactness for
    arbitrary conditioned floats is restored host-side with np.where.
"""

import os
import sys
import numpy as np

for _p in ("/opt/trn_rl_repo", "/root/.axon_site/_ro/trn_rl_repo"):
    if _p not in sys.path and os.path.isdir(_p):
        sys.path.append(_p)

N, C, H, W, K, KS = 64, 3, 12, 12, 128, 5
HW = H * W
NCORES = 8
NLOC = N // NCORES  # 8 samples per core
PH, PW = H + 2, W + 4  # padded buffer: 2 pad rows on top, 2 pad cols each side
NPOS = PH * PW  # 224
KY = 3  # only kernel rows 0..2 can be nonzero under the type-A mask
RED = C * KY * KS  # 45 contraction rows: r = c_in*15 + ky*5 + kx
BIG = 1e30


def _mask_np():
    m = np.ones((KS, KS), np.float32)
    c = KS // 2
    m[c, c:] = 0.0
    m[c + 1:, :] = 0.0
    return m


def _host_tables(conditioned_on, Wc, b):
    """Build weight matrix [45, 384] and per-core noise tables [24, HW*K]."""
    import jax
    import jax.numpy as jnp

    Wm = np.asarray(Wc, np.float32) * _mask_np()[None, None]  # [384,3,5,5]
    # rhs table: rows r = ky*15 + c_in*5 + kx, cols j = c_out*K + k ;
    # value Wm[k*3+c_out, c_in, ky, kx]
    Wt = Wm[:, :, 0:KY, :].transpose(0, 2, 1, 3).reshape(K * C, RED)  # [kc, 45]
    Wr = np.ascontiguousarray(
        Wt.reshape(K, C, RED).transpose(2, 1, 0).reshape(RED, C * K))  # [45, 384]

    # Gumbel noise, bit-exact with jax.random.categorical on CPU.
    cpu = jax.devices("cpu")[0]
    with jax.default_device(cpu):
        base = jax.random.key(42)

        def g(i):
            return jax.random.gumbel(jax.random.fold_in(base, i), (N, K, C),
                                     jnp.float32)

        gj = jax.jit(g)
        G = np.stack([np.asarray(gj(i)) for i in range(HW)])  # [HW, N, K, C]

    b_kc = np.asarray(b, np.float32).reshape(K, C)
    A = G + b_kc[None, None]  # [HW, N, K, C]
    At = np.ascontiguousarray(A.transpose(1, 3, 0, 2))  # [N, C, HW, K]

    cf = np.asarray(conditioned_on, np.float32).reshape(N, C, HW)
    condm = cf >= 0  # conditioned (kept) pixels
    if condm.any():
        kforce = np.clip(np.rint(cf), 0, K - 1).astype(np.int64)  # [N,C,HW]
        onehot = np.arange(K)[None, None, None, :] == kforce[..., None]
        At = np.where(condm[..., None], np.where(onehot, BIG, -BIG), At)

    noise_cores = []
    for d in range(NCORES):
        nd = At[d * NLOC:(d + 1) * NLOC]  # [8, C, HW, K]
        packed = nd.transpose(1, 0, 2, 3).reshape(C, NLOC, HW * K)  # [co, nl, HW*K]
        # place block co at partitions co*32..co*32+7; rest zero (the add /
        # max run over the full 96-partition tile, must be initialized)
        n96 = np.zeros((C * 32, HW * K), np.float32)
        for co in range(C):
            n96[co * 32:co * 32 + NLOC] = packed[co]
        noise_cores.append(n96)
    return Wr, noise_cores


def build_program(debug=False):
    import concourse.bass as bass
    from concourse import bacc, mybir, tile
    from concourse.tile import add_dep_helper

    f32 = mybir.dt.float32
    nc = bacc.Bacc("TRN2", target_bir_lowering=False, debug=debug)
    P96 = C * 32
    wr_d = nc.dram_tensor("wr", [RED, C * K], f32, kind="ExternalInput")
    nz_d = nc.dram_tensor("noise", [P96, HW * K], f32, kind="ExternalInput")
    out_d = nc.dram_tensor("out", [NLOC, C, NPOS], f32, kind="ExternalOutput")

    with tile.TileContext(nc) as tc:
        with (
            tc.tile_pool(name="const", bufs=1) as cpool,
            tc.tile_pool(name="work", bufs=4) as pool,
            tc.tile_pool(name="ps", bufs=4, space="PSUM") as pspool,
        ):
            wr = cpool.tile([RED, C * K], f32, tag="wr")
            img = cpool.tile([C, NPOS * NLOC], f32, tag="img")
            nz = cpool.tile([P96, HW * K], f32, tag="nz")
            patch = cpool.tile([RED, 32], f32, tag="patch")
            nc.sync.dma_start(out=wr[:, :], in_=wr_d[:, :])
            nc.vector.memset(img[:, :], 0.0)
            nc.vector.memset(patch[:, :], 0.0)
            # chunked load of the noise table to spread across DMA queues
            NCH = 8
            chw = (HW * K) // NCH
            for ch in range(NCH):
                nc.sync.dma_start(out=nz[:, ch * chw:(ch + 1) * chw],
                                  in_=nz_d[:, ch * chw:(ch + 1) * chw])

            for i in range(HW):
                y, x = divmod(i, W)
                # window top-left in padded buf = (y-2+2, x-2+2) = (y, x)
                imv = img[:, :].rearrange("c (yy xx n) -> c yy xx n",
                                          yy=PH, xx=PW)
                # one DMA per kernel row: patch rows ky*15 + (c_in, kx);
                # cols 8..31 stay zero so M=32 matmuls write zeros there
                for ky in range(KY):
                    nc.sync.dma_start(
                        out=patch[ky * (C * KS):(ky + 1) * (C * KS), 0:NLOC],
                        in_=imv[:, y + ky, x:x + KS, :])

                ps = pspool.tile([P96, K], f32, tag="ps")
                for co in range(C):
                    nc.tensor.matmul(
                        ps[co * 32:(co + 1) * 32, :],
                        patch[:, :],
                        wr[:, co * K:(co + 1) * K],
                        start=True, stop=True,
                    )
                lg = pool.tile([P96, K], f32, tag="lg")
                nc.vector.tensor_tensor(
                    out=lg[:, :], in0=ps[:, :], in1=nz[:, i * K:(i + 1) * K],
                    op=mybir.AluOpType.add)
                m8 = pool.tile([P96, 8], f32, tag="m8")
                nc.vector.max(m8[:, :], lg[:, :])
                idx = pool.tile([P96, 8], mybir.dt.uint32, tag="idx")
                nc.vector.max_index(idx[:, :], m8[:, :], lg[:, :])
                nv = pool.tile([P96, 1], f32, tag="nv")
                cast_i = nc.vector.tensor_copy(nv[:, :], idx[:, 0:1])

                q = (y + 2) * PW + (x + 2)
                for co in range(C):
                    nc.sync.dma_start(
                        out=img[co:co + 1, q * NLOC:(q + 1) * NLOC],
                        in_=nv[co * 32:co * 32 + NLOC, :])

            # ship the whole padded buffer; host slices the interior
            nc.sync.dma_start(
                out=out_d[:, :, :].rearrange("n c q -> c q n"),
                in_=img[:, :].rearrange("c (q n) -> c q n", n=NLOC))

    nc.compile()
    return nc


_prog_cache = {}


def _get_prog():
    if "nc" not in _prog_cache:
        _prog_cache["nc"] = build_program(debug=False)
    return _prog_cache["nc"]


def kernel(conditioned_on, Wc, b):
    from concourse.bass_utils import run_bass_kernel_spmd

    conditioned_on = np.asarray(conditioned_on, np.float32)
    Wr, noise_cores = _host_tables(conditioned_on, Wc, b)
    nc = _get_prog()
    in_maps = [{"wr": Wr, "noise": noise_cores[d]} for d in range(NCORES)]
    res = run_bass_kernel_spmd(nc, in_maps, core_ids=list(range(NCORES)))
    out = np.concatenate([res.results[d]["out"] for d in range(NCORES)], axis=0)
    out = out.reshape(N, C, PH, PW)[:, :, 2:2 + H, 2:2 + W]
    out = np.ascontiguousarray(out).astype(np.float32)
    # exact passthrough for conditioned pixels (handles non-integer values)
    out = np.where(conditioned_on >= 0, conditioned_on, out)
    return out


# revision 22
# speedup vs baseline: 1.5679x; 1.5679x over previous
"""Trainium2 Bass kernel for autoregressive masked-conv sampling.

Model: type-A masked 5x5 conv [n,C,H,W] -> [n,K,C,H,W] logits; per-pixel
raster-order categorical sampling over K with jax threefry gumbel noise.

Strategy:
  - Gumbel noise is data-independent => precompute on host CPU with jax,
    bit-matching jax.random.categorical's internals.
  - Data-parallel over batch n=64 across 8 cores (8 samples/core).
  - On device, per pixel i: gather 3x5 (x3 in-ch) padded window -> [45,8]
    patchT, 3 matmuls (one per conv in-group c_out) -> PSUM [24,128]
    (partition p = c_out*8 + n_local), add noise slice, DVE max/max_index
    argmax over K, cast to f32, scatter the sampled value back into the
    padded image buffer. 144 strictly sequential steps.
  - Conditioned pixels (cond >= 0) are forced via +/-1e30 noise so the
    argmax returns the conditioned (integer) value; final exactness for
    arbitrary conditioned floats is restored host-side with np.where.
"""

import os
import sys
import numpy as np

for _p in ("/opt/trn_rl_repo", "/root/.axon_site/_ro/trn_rl_repo"):
    if _p not in sys.path and os.path.isdir(_p):
        sys.path.append(_p)

N, C, H, W, K, KS = 64, 3, 12, 12, 128, 5
HW = H * W
NCORES = 8
NLOC = N // NCORES  # 8 samples per core
PH, PW = H + 2, W + 4  # padded buffer: 2 pad rows on top, 2 pad cols each side
NPOS = PH * PW  # 224
KY = 3  # only kernel rows 0..2 can be nonzero under the type-A mask
RED = C * KY * KS  # 45 contraction rows: r = c_in*15 + ky*5 + kx
BIG = 1e30


def _mask_np():
    m = np.ones((KS, KS), np.float32)
    c = KS // 2
    m[c, c:] = 0.0
    m[c + 1:, :] = 0.0
    return m


def _host_tables(conditioned_on, Wc, b):
    """Build weight matrix [45, 384] and per-core noise tables [24, HW*K]."""
    import jax
    import jax.numpy as jnp

    Wm = np.asarray(Wc, np.float32) * _mask_np()[None, None]  # [384,3,5,5]
    # rhs table: rows r = ky*15 + c_in*5 + kx, cols j = c_out*K + k ;
    # value Wm[k*3+c_out, c_in, ky, kx]
    Wt = Wm[:, :, 0:KY, :].transpose(0, 2, 1, 3).reshape(K * C, RED)  # [kc, 45]
    Wr = np.ascontiguousarray(
        Wt.reshape(K, C, RED).transpose(2, 1, 0).reshape(RED, C * K))  # [45, 384]

    # Gumbel noise, bit-exact with jax.random.categorical on CPU.
    cpu = jax.devices("cpu")[0]
    with jax.default_device(cpu):
        base = jax.random.key(42)

        def g(i):
            return jax.random.gumbel(jax.random.fold_in(base, i), (N, K, C),
                                     jnp.float32)

        gj = jax.jit(g)
        G = np.stack([np.asarray(gj(i)) for i in range(HW)])  # [HW, N, K, C]

    b_kc = np.asarray(b, np.float32).reshape(K, C)
    A = G + b_kc[None, None]  # [HW, N, K, C]
    At = np.ascontiguousarray(A.transpose(1, 3, 0, 2))  # [N, C, HW, K]

    cf = np.asarray(conditioned_on, np.float32).reshape(N, C, HW)
    condm = cf >= 0  # conditioned (kept) pixels
    if condm.any():
        kforce = np.clip(np.rint(cf), 0, K - 1).astype(np.int64)  # [N,C,HW]
        onehot = np.arange(K)[None, None, None, :] == kforce[..., None]
        At = np.where(condm[..., None], np.where(onehot, BIG, -BIG), At)

    noise_cores = []
    for d in range(NCORES):
        nd = At[d * NLOC:(d + 1) * NLOC]  # [8, C, HW, K]
        packed = nd.transpose(1, 0, 2, 3).reshape(C * NLOC, HW * K)
        noise_cores.append(np.ascontiguousarray(packed))
    return Wr, noise_cores


def build_program(debug=False):
    import concourse.bass as bass
    from concourse import bacc, mybir, tile
    from concourse.tile import add_dep_helper

    f32 = mybir.dt.float32
    nc = bacc.Bacc("TRN2", target_bir_lowering=False, debug=debug)
    P96 = C * 32
    wr_d = nc.dram_tensor("wr", [RED, C * K], f32, kind="ExternalInput")
    nz_d = nc.dram_tensor("noise", [C * NLOC, HW * K], f32, kind="ExternalInput")
    out_d = nc.dram_tensor("out", [NLOC, C, NPOS], f32, kind="ExternalOutput")

    with tile.TileContext(nc) as tc:
        with (
            tc.tile_pool(name="const", bufs=1) as cpool,
            tc.tile_pool(name="work", bufs=4) as pool,
            tc.tile_pool(name="ps", bufs=4, space="PSUM") as pspool,
        ):
            wr = cpool.tile([RED, C * K], f32, tag="wr")
            img = cpool.tile([C, NPOS * NLOC], f32, tag="img")
            nz = cpool.tile([P96, HW * K], f32, tag="nz")
            patch = cpool.tile([RED, 32], f32, tag="patch")
            nc.sync.dma_start(out=wr[:, :], in_=wr_d[:, :])
            nc.vector.memset(img[:, :], 0.0)
            nc.vector.memset(patch[:, :], 0.0)
            # expand compact noise rows to partitions co*32..co*32+7, chunked
            # across DMA queues; zero the in-between rows (the full-height
            # add reads them) on three engines in parallel
            # zero whole tile first; the noise DMAs then overwrite the 3
            # valid row blocks (Tile orders the overlapping writes)
            nc.gpsimd.memset(nz[:, :], 0.0)
            NCH = 4
            chw = (HW * K) // NCH
            for co in range(C):
                for ch in range(NCH):
                    sl = slice(ch * chw, (ch + 1) * chw)
                    nc.sync.dma_start(out=nz[co * 32:co * 32 + NLOC, sl],
                                      in_=nz_d[co * NLOC:(co + 1) * NLOC, sl])

            for i in range(HW):
                y, x = divmod(i, W)
                # window top-left in padded buf = (y-2+2, x-2+2) = (y, x)
                imv = img[:, :].rearrange("c (yy xx n) -> c yy xx n",
                                          yy=PH, xx=PW)
                # one DMA per kernel row: patch rows ky*15 + (c_in, kx);
                # cols 8..31 stay zero so M=32 matmuls write zeros there
                for ky in range(KY):
                    nc.sync.dma_start(
                        out=patch[ky * (C * KS):(ky + 1) * (C * KS), 0:NLOC],
                        in_=imv[:, y + ky, x:x + KS, :])

                ps = pspool.tile([P96, K], f32, tag="ps")
                for co in range(C):
                    nc.tensor.matmul(
                        ps[co * 32:(co + 1) * 32, :],
                        patch[:, :],
                        wr[:, co * K:(co + 1) * K],
                        start=True, stop=True,
                    )
                lg = pool.tile([P96, K], f32, tag="lg")
                nc.vector.tensor_tensor(
                    out=lg[:, :], in0=ps[:, :], in1=nz[:, i * K:(i + 1) * K],
                    op=mybir.AluOpType.add)
                m8 = pool.tile([P96, 8], f32, tag="m8")
                nc.vector.max(m8[:, :], lg[:, :])
                idx = pool.tile([P96, 8], mybir.dt.uint32, tag="idx")
                nc.vector.max_index(idx[:, :], m8[:, :], lg[:, :])
                nv = pool.tile([P96, 1], f32, tag="nv")
                cast_i = nc.vector.tensor_copy(nv[:, :], idx[:, 0:1])

                q = (y + 2) * PW + (x + 2)
                for co in range(C):
                    nc.sync.dma_start(
                        out=img[co:co + 1, q * NLOC:(q + 1) * NLOC],
                        in_=nv[co * 32:co * 32 + NLOC, :])

            # ship the whole padded buffer; host slices the interior
            nc.sync.dma_start(
                out=out_d[:, :, :].rearrange("n c q -> c q n"),
                in_=img[:, :].rearrange("c (q n) -> c q n", n=NLOC))

    nc.compile()
    return nc


_prog_cache = {}


def _get_prog():
    if "nc" not in _prog_cache:
        _prog_cache["nc"] = build_program(debug=False)
    return _prog_cache["nc"]


def kernel(conditioned_on, Wc, b):
    from concourse.bass_utils import run_bass_kernel_spmd

    conditioned_on = np.asarray(conditioned_on, np.float32)
    Wr, noise_cores = _host_tables(conditioned_on, Wc, b)
    nc = _get_prog()
    in_maps = [{"wr": Wr, "noise": noise_cores[d]} for d in range(NCORES)]
    res = run_bass_kernel_spmd(nc, in_maps, core_ids=list(range(NCORES)))
    out = np.concatenate([res.results[d]["out"] for d in range(NCORES)], axis=0)
    out = out.reshape(N, C, PH, PW)[:, :, 2:2 + H, 2:2 + W]
    out = np.ascontiguousarray(out).astype(np.float32)
    # exact passthrough for conditioned pixels (handles non-integer values)
    out = np.where(conditioned_on >= 0, conditioned_on, out)
    return out
